# revision 1
# baseline (speedup 1.0000x reference)
"""AFT-Full forward on 8 Trainium2 NeuronCores (Bass/Tile, SPMD).

Reference (per batch b):
    Q = x^T wq^T + bq ; K = x^T wk^T + bk ; V = x^T wv^T + bv      # [T, H]
    ew = exp(wbias[:T, :T])                                        # [T, T]
    num = ew @ (exp(K) * V) ; den = ew @ exp(K)                    # [T, H]
    out = (sigmoid(Q) * num / den) @ wp^T + bp                     # [T, DIM]

Sharding (2-D): 2 batch-groups x 4 query-time slices.
Core c -> (h = c // 4 batch-group, g = c % 4 t-slice of 1024 rows).
 - core c computes K/V (and Z = [exp(K)*V | exp(K)]) for its OWN batch
   b = 4h + g over the full sequence; a chain of pipelined 4-rank
   AllGathers (one per sequence split, so the main matmuls overlap the
   later gathers) shares Z with the batch-group.
 - core c then produces out[4h:4h+4, g*1024:(g+1)*1024, :] using only
   wbias rows [g*1024, (g+1)*1024)  (host pre-transposes them to [s, t]).

Z travels partition-major ([128, s*128+h] rather than [s, h]) so every
DMA touching it moves contiguous multi-KB rows; the Z writes go through
SWDGE (gpsimd) so they are not queued behind the big HWDGE streams.

All matmul operands are bf16 (PE runs fp32 matmul at 1/4 rate); all
accumulation is fp32 in PSUM; exp/divide run in fp32. Sigmoid is
computed as 1/(1+exp(-Q-bq)) on the Exp LUT so the scalar engine never
reloads activation tables. Measured end-to-end relative error vs the
fp32 reference ~4.5e-3.

Biases: bq is folded into the exp activation (host passes -bq); bp is
folded into the output projection via an appended ones-row; bk/bv are
folded into the K/V matmul as a rank-1 (K=1) accumulation step.
"""

import numpy as np
import ml_dtypes

B, DIM, T, H = 8, 512, 4096, 64
H2 = 2 * H
NCORES = 8
NG = 2            # batch groups
G = 4             # t-slices per group (ranks per collective group)
NB = B // NG      # batches per group = 4
TSL = T // G      # 1024 t rows per core
SCH = T // 128    # 32 s-chunks
SPLITS = [10, 12, 10]    # pipelined AllGather split sizes (s-chunks)
SOFF = [0]
for _n in SPLITS:
    SOFF.append(SOFF[-1] + _n)
NSP = len(SPLITS)
DCH = DIM // 128  # 4 contraction chunks for projections

BF16 = ml_dtypes.bfloat16

_CACHE = {}
RUN_KWARGS = {}        # test harness may set {"trace": True}
LAST_RESULT = [None]   # test harness reads exec_time_ns off this


def _build():
    import concourse.mybir as mybir
    import concourse.tile as tile
    from concourse import bacc

    fp32 = mybir.dt.float32
    bf16 = mybir.dt.bfloat16
    AF = mybir.ActivationFunctionType

    nc = bacc.Bacc("TRN2", target_bir_lowering=False, debug=False,
                   num_devices=NCORES)

    xb_ext = nc.dram_tensor("xb", [DIM, T], bf16, kind="ExternalInput").ap()
    xq_ext = nc.dram_tensor("xq", [NB, DIM, TSL], bf16, kind="ExternalInput").ap()
    ewb_ext = nc.dram_tensor("ewb", [T, TSL], bf16, kind="ExternalInput").ap()
    wkv_ext = nc.dram_tensor("wkv", [DIM, H2], bf16, kind="ExternalInput").ap()
    wqt_ext = nc.dram_tensor("wqt", [DIM, H], bf16, kind="ExternalInput").ap()
    wpta_ext = nc.dram_tensor("wpta", [H + 1, DIM], bf16, kind="ExternalInput").ap()
    bq_ext = nc.dram_tensor("bq", [H, 1], fp32, kind="ExternalInput").ap()
    bkv_ext = nc.dram_tensor("bkv", [1, H2], bf16, kind="ExternalInput").ap()
    out_ext = nc.dram_tensor("out", [NB, TSL, DIM], bf16, kind="ExternalOutput").ap()

    groups = [list(range(0, G)), list(range(G, 2 * G))]

    with tile.TileContext(nc) as tc:
        with (
            tc.tile_pool(name="const", bufs=1) as cpool,
            tc.tile_pool(name="res", bufs=1) as rpool,
            tc.tile_pool(name="work", bufs=3) as wpool,
            tc.tile_pool(name="dram", bufs=1, space="DRAM") as dpool,
        ):
            # ---- constants ----
            wkv_sb = cpool.tile([128, DCH, H2], bf16)
            nc.sync.dma_start(wkv_sb[:], wkv_ext.rearrange("(n p) m -> p n m", p=128))
            wqt_sb = cpool.tile([128, DCH, H], bf16)
            nc.sync.dma_start(wqt_sb[:], wqt_ext.rearrange("(n p) m -> p n m", p=128))
            wpta_sb = cpool.tile([H + 1, DIM], bf16)
            nc.sync.dma_start(wpta_sb[:], wpta_ext[:])
            bq_sb = cpool.tile([H, 1], fp32)
            nc.sync.dma_start(bq_sb[:], bq_ext[:])
            bkv_sb = cpool.tile([1, H2], bf16)
            nc.sync.dma_start(bkv_sb[:], bkv_ext[:])
            ones_sb = cpool.tile([1, 128], bf16)
            nc.vector.memset(ones_sb[:], 1.0)

            # ---- resident tensors; big HWDGE streams issue up front ----
            xb_sb = rpool.tile([128, DCH, T], bf16)       # 8KB/part
            for d in range(DCH):
                nc.sync.dma_start(xb_sb[:, d, :],
                                  xb_ext[d * 128:(d + 1) * 128, :])
            xq_sbs = [wpool.tile([128, DCH, TSL], bf16, tag="xq", bufs=2,
                                 name=f"xq_sb{bl}") for bl in range(NB)]
            for bl in range(2):   # bl2/3 stream later (slot rotation)
                nc.sync.dma_start(
                    xq_sbs[bl][:],
                    xq_ext[bl].rearrange("(n p) t -> p n t", p=128))
            ew_sb = rpool.tile([128, SCH, TSL], bf16)     # exp(wbias^T), 64KB/part
            for s in range(SCH):
                nc.sync.dma_start(ew_sb[:, s, :],
                                  ewb_ext[s * 128:(s + 1) * 128, :])
            z_acc = rpool.tile([128, SCH, H2], bf16)      # own-batch Z, p-major
            z_res = [rpool.tile([128, NB, SPLITS[k], H2], bf16,
                                name=f"z_res{k}")
                     for k in range(NSP)]                 # gathered Z, 32KB/part
            sq_sb = rpool.tile([H, NB, TSL], fp32)        # sigmoid(Q^T)

            z_own = [dpool.tile([128, SPLITS[i] * H2], bf16, name=f"z_own{i}")
                     for i in range(NSP)]
            z_all = [dpool.tile([G * 128, SPLITS[i] * H2], bf16,
                                name=f"z_all{i}")
                     for i in range(NSP)]

            # ---- phase 1: Z for own batch; AllGather per half ----
            with tc.tile_pool(name="ps_a", bufs=2, space="PSUM") as ps_a:
                for k in range(NSP):
                    for sl in range(SPLITS[k]):
                        s = SOFF[k] + sl
                        kv_ps = ps_a.tile([128, H2], fp32, tag="kv", bufs=4)
                        for d in range(DCH):
                            nc.tensor.matmul(
                                kv_ps[:], xb_sb[:, d, s * 128:(s + 1) * 128],
                                wkv_sb[:, d, :], start=(d == 0), stop=False)
                        # rank-1 bias fold: += ones^T @ [bv | bk]
                        nc.tensor.matmul(kv_ps[:], ones_sb[:], bkv_sb[:],
                                         start=False, stop=True)
                        ek_sb = wpool.tile([128, H], fp32, tag="ek")
                        nc.scalar.activation(ek_sb[:], kv_ps[:, H:H2], AF.Exp)
                        nc.vector.tensor_mul(z_acc[:, s, 0:H], kv_ps[:, 0:H],
                                             ek_sb[:])
                        nc.vector.tensor_copy(z_acc[:, s, H:H2], ek_sb[:])
                    # SWDGE so this is not queued behind the HWDGE streams
                    nc.gpsimd.dma_start(
                        z_own[k][:],
                        z_acc[:, SOFF[k]:SOFF[k + 1], :])
                    nc.gpsimd.collective_compute(
                        "AllGather", mybir.AluOpType.bypass,
                        replica_groups=groups,
                        ins=[z_own[k].opt()], outs=[z_all[k].opt()],
                    )

                # ---- phase 2a: sigmoid(Q) (PE idles during the gathers) ----
                for bl in range(NB):
                    xq_sb = xq_sbs[bl]
                    if bl >= 2:
                        nc.sync.dma_start(
                            xq_sb[:],
                            xq_ext[bl].rearrange("(n p) t -> p n t", p=128),
                        )
                    for th in range(TSL // 512):
                        q_ps = ps_a.tile([H, 512], fp32, tag="q")
                        for d in range(DCH):
                            nc.tensor.matmul(
                                q_ps[:], wqt_sb[:, d, :],
                                xq_sb[:, d, th * 512:(th + 1) * 512],
                                start=(d == 0), stop=(d == DCH - 1))
                        # sigmoid via the Exp LUT (avoids table reloads):
                        # sq = 1 / (1 + exp(-Q - bq));  bq_ext holds -bq.
                        eq_sb = wpool.tile([H, 512], fp32, tag="eq")
                        nc.scalar.activation(eq_sb[:], q_ps[:], AF.Exp,
                                             bias=bq_sb[:], scale=-1.0)
                        nc.vector.tensor_scalar_add(eq_sb[:], eq_sb[:], 1.0)
                        nc.vector.reciprocal_approx_fast(
                            sq_sb[:, bl, th * 512:(th + 1) * 512], eq_sb[:])

                # ---- phase 2b: ew = exp(wbias^T) in place (after the
                # sigmoid exps in ACT order, so the Q chain finishes early
                # and releases its PSUM banks before the main matmuls) ----
                for s in range(SCH):
                    nc.scalar.activation(ew_sb[:, s, :], ew_sb[:, s, :], AF.Exp)

                # ---- gathered Z -> SBUF resident (per split, per batch) ----
                for k in range(NSP):
                    for bl in range(NB):
                        nc.sync.dma_start(
                            z_res[k][:, bl, :, :],
                            z_all[k][bl * 128:(bl + 1) * 128, :]
                            .rearrange("p (s h) -> p s h", s=SPLITS[k]),
                        )

            # ---- phase 3: num/den matmuls + epilogue ----
            # Single PSUM tag: 4 live [128,1024] accumulators (8 banks);
            # the oproj tiles reuse the slots as the accumulators retire.
            def epilogue_dve(nd_ps, bl, yt_sb):
                # reciprocal_approx_* are custom DVE ops — feed them
                # from SBUF, not PSUM (PSUM reads gave garbage).
                den_sb = wpool.tile([H, TSL], fp32, tag="den", bufs=1,
                                    name=f"den{bl}")
                nc.scalar.copy(den_sb[:], nd_ps[H:H2, :])
                rcp_sb = wpool.tile([H, TSL], fp32, tag="rcp", bufs=1,
                                    name=f"rcp{bl}")
                nc.vector.reciprocal_approx_fast(rcp_sb[:], den_sb[:])
                nc.vector.tensor_mul(yt_sb[0:H, :], nd_ps[0:H, :], rcp_sb[:])
                nc.vector.tensor_mul(yt_sb[0:H, :], yt_sb[0:H, :],
                                     sq_sb[:, bl, :])
                nc.vector.memset(yt_sb[H:H + 1, :], 1.0)

            def oproj(bl, yt_sb, ps_b, last=False):
                for tch in range(TSL // 128):
                    o_ps = ps_b.tile([128, DIM], fp32, tag="mn", bufs=4,
                                     name=f"o_ps{bl}_{tch}")
                    nc.tensor.matmul(
                        o_ps[:], yt_sb[:, tch * 128:(tch + 1) * 128],
                        wpta_sb[:], start=True, stop=True)
                    o_sb = wpool.tile([128, DIM], bf16, tag="o", bufs=4,
                                      name=f"o_sb{bl}_{tch}")
                    if tch % 2 == 0:
                        nc.vector.tensor_copy(o_sb[:], o_ps[:])
                    else:
                        nc.scalar.copy(o_sb[:], o_ps[:])
                    nc.sync.dma_start(
                        out_ext[bl, tch * 128:(tch + 1) * 128, :], o_sb[:])

            with tc.tile_pool(name="ps_b", bufs=1, space="PSUM") as ps_b:
                nd_pss = [ps_b.tile([128, TSL], fp32, tag="mn", bufs=4,
                                    name=f"nd_ps{bl}")
                          for bl in range(NB)]

                def mm(bl, s):
                    k = next(i for i in range(NSP) if SOFF[i + 1] > s)
                    sl = s - SOFF[k]
                    for th in range(TSL // 512):
                        nc.tensor.matmul(
                            nd_pss[bl][:, th * 512:(th + 1) * 512],
                            z_res[k][:, bl, sl, :],
                            ew_sb[:, s, th * 512:(th + 1) * 512],
                            start=(s == 0), stop=(s == SCH - 1))

                # splits 0..NSP-2: all batches (overlap the later gathers)
                for s in range(SOFF[NSP - 1]):
                    for bl in range(NB):
                        mm(bl, s)
                # last split: stagger per batch so epilogues pipeline
                yt_sbs = [None] * NB
                for bl in range(NB):
                    for s in range(SOFF[NSP - 1], SCH):
                        mm(bl, s)
                    yt_sbs[bl] = wpool.tile([H + 1, TSL], bf16, tag="yt",
                                            bufs=2, name=f"yt{bl}")
                    epilogue_dve(nd_pss[bl], bl, yt_sbs[bl])
                    if bl >= 1:
                        oproj(bl - 1, yt_sbs[bl - 1], ps_b)
                oproj(NB - 1, yt_sbs[NB - 1], ps_b, last=True)

    nc.compile()
    return nc


def _get_nc():
    if "nc" not in _CACHE:
        _CACHE["nc"] = _build()
    return _CACHE["nc"]


def kernel(x, wq, bq, wk, bk, wv, bv, wp, bp, wbias):
    from concourse.bass_utils import run_bass_kernel_spmd

    x = np.asarray(x); wbias = np.asarray(wbias)
    wkv = np.concatenate([np.asarray(wv).T, np.asarray(wk).T], axis=1).astype(BF16)
    bkv = np.concatenate([np.asarray(bv), np.asarray(bk)])[None, :].astype(BF16)
    wqt = np.asarray(wq).T.astype(BF16)
    wpta = np.concatenate([np.asarray(wp).T, np.asarray(bp)[None, :]],
                          axis=0).astype(BF16)
    bq_in = (-np.asarray(bq)).reshape(H, 1).astype(np.float32)

    in_maps = []
    for c in range(NCORES):
        g, h = c % G, c // G
        tsl = slice(g * TSL, (g + 1) * TSL)
        in_maps.append({
            "xb": x[G * h + g].astype(BF16),
            "xq": x[NB * h:NB * (h + 1), :, tsl].astype(BF16),
            "ewb": np.ascontiguousarray(wbias[tsl, :].T).astype(BF16),
            "wkv": wkv, "wqt": wqt, "wpta": wpta, "bq": bq_in, "bkv": bkv,
        })

    nc = _get_nc()
    res = run_bass_kernel_spmd(nc, in_maps, core_ids=list(range(NCORES)),
                               **RUN_KWARGS)
    LAST_RESULT[0] = res

    out_full = np.empty((B, T, DIM), np.float32)
    for c in range(NCORES):
        g, h = c % G, c // G
        out_full[NB * h:NB * (h + 1), g * TSL:(g + 1) * TSL, :] = \
            res.results[c]["out"].astype(np.float32)
    return (out_full, out_full)



# revision 6
# speedup vs baseline: 1.0578x; 1.0578x over previous
"""AFT-Full forward on 8 Trainium2 NeuronCores (Bass/Tile, SPMD).

Reference (per batch b):
    Q = x^T wq^T + bq ; K = x^T wk^T + bk ; V = x^T wv^T + bv      # [T, H]
    ew = exp(wbias[:T, :T])                                        # [T, T]
    num = ew @ (exp(K) * V) ; den = ew @ exp(K)                    # [T, H]
    out = (sigmoid(Q) * num / den) @ wp^T + bp                     # [T, DIM]

Sharding: one batch per core (B == NCORES) -- zero collectives.  Each
core loads its full x (bf16, 4MB), streams the full T x T weight matrix
in fp8 (16MB) and writes its out (bf16, 4MB).  The kernel is a pure
DMA-paced stream with no inter-core dependency.

Numerics: ew = exp(wbias) = 1 + expm1(wbias).  The host sends
ewm1 = expm1(wbias)^T * 4096 as float8_e4m3; the rank-1 "ones" part is
applied as colsum = sum_s Z[s,:] computed on-chip from bf16 Z in fp32
and added into the same PSUM accumulation via two bf16 rank-1 matmuls
(hi + lo split of colsum, rhs = a row of 4096.0).  Both operands of the
big matmul are fp8 -> MatmulPerfMode.DoubleRow packs two s-chunks per
instruction (~1.5-2x PE).  Because all the precision-critical mass is
in the colsum term, fp8 quantization of ewm1/Z only perturbs the small
deviation part: CPU-validated end-to-end rel err ~4.0e-3 (the bf16
baseline scheme measures ~4.2e-3).

The num/den ratio cancels the 4096 scale, so no descaling is needed.
Sigmoid is computed as 1/(1+exp(-Q-bq)) on the Exp LUT so the scalar
engine never reloads activation tables.  bkv is folded into the K/V
matmul as a rank-1 accumulation; bp via an appended ones-row in the
output projection.

DMA plan: ew pairs stream on the sync (SP) HWDGE ring in consumption
order; x blocks + out chunks ride the scalar (ACT) HWDGE ring so the
two streams overlap at the HBM controller.
"""

import numpy as np
import ml_dtypes

B, DIM, T, H = 8, 512, 4096, 64
H2 = 2 * H
NCORES = 8
DCH = DIM // 128    # 4 contraction chunks
SCH = T // 128      # 32 s-chunks
NTB = T // 512      # 8 t-blocks for x / Q
NPAIR = SCH // 2    # 16 s-chunk pairs (DoubleRow)
NQT = 4             # t-quarters (DoubleRow PSUM outs must sit at
TQ = T // NQT       # partition base 0 -> separate [64, 1024] num/den)
SC = 4096.0         # fp8 scale for ewm1 (power of 2; cancels in num/den)

BF16 = ml_dtypes.bfloat16
F8 = ml_dtypes.float8_e4m3

_CACHE = {}
RUN_KWARGS = {}        # test harness may set {"trace": True}
LAST_RESULT = [None]   # test harness reads exec_time_ns off this


def _build():
    import concourse.mybir as mybir
    import concourse.tile as tile
    from concourse import bacc

    fp32 = mybir.dt.float32
    bf16 = mybir.dt.bfloat16
    fp8 = mybir.dt.float8e4
    AF = mybir.ActivationFunctionType
    DR = mybir.MatmulPerfMode.DoubleRow

    nc = bacc.Bacc("TRN2", target_bir_lowering=False, debug=False,
                   num_devices=NCORES)

    xb_ext = nc.dram_tensor("xb", [128, NTB, DCH, 512], bf16,
                            kind="ExternalInput").ap()
    ewb_ext = nc.dram_tensor("ewb", [NQT, NPAIR, 128, 2 * TQ], fp8,
                             kind="ExternalInput").ap()
    wkv_ext = nc.dram_tensor("wkv", [128, DCH, H2], bf16,
                             kind="ExternalInput").ap()
    wqt_ext = nc.dram_tensor("wqt", [128, DCH, H], bf16,
                             kind="ExternalInput").ap()
    wpta_ext = nc.dram_tensor("wpta", [H + 1, DIM], bf16,
                              kind="ExternalInput").ap()
    bkv_ext = nc.dram_tensor("bkv", [1, H2], bf16, kind="ExternalInput").ap()
    bqn_ext = nc.dram_tensor("bqn", [H, 1], fp32, kind="ExternalInput").ap()
    out_ext = nc.dram_tensor("out", [T, DIM], bf16, kind="ExternalOutput").ap()

    with tile.TileContext(nc) as tc:
        with (
            tc.tile_pool(name="const", bufs=1) as cpool,
            tc.tile_pool(name="res", bufs=1) as rpool,
            tc.tile_pool(name="work", bufs=2) as wpool,
            tc.tile_pool(name="ew", bufs=1) as epool,
        ):
            # ---- constants (sync ring; tiny) ----
            wkv_sb = cpool.tile([128, DCH, H2], bf16)
            nc.sync.dma_start(wkv_sb[:], wkv_ext[:])
            wqt_sb = cpool.tile([128, DCH, H], bf16)
            nc.sync.dma_start(wqt_sb[:], wqt_ext[:])
            wpta_sb = cpool.tile([H + 1, DIM], bf16)
            nc.sync.dma_start(wpta_sb[:], wpta_ext[:])
            bkv_sb = cpool.tile([1, H2], bf16)
            nc.sync.dma_start(bkv_sb[:], bkv_ext[:])
            bqn_sb = cpool.tile([H, 1], fp32)
            nc.sync.dma_start(bqn_sb[:], bqn_ext[:])
            ones_sb = cpool.tile([1, 128], bf16)     # bkv rank-1 lhsT
            nc.vector.memset(ones_sb[:], 1.0)
            onesc_sb = cpool.tile([128, 1], bf16)    # colsum lhsT
            nc.vector.memset(onesc_sb[:], 1.0)
            scrow_sb = cpool.tile([1, 512], bf16)    # colsum rank-1 rhs (= SC)
            nc.vector.memset(scrow_sb[:], SC)

            # ---- x blocks on the scalar ring (overlap the ew stream) ----
            x_tbs = []
            for tb in range(NTB):
                x_tb = rpool.tile([128, DCH, 512], bf16, name=f"x{tb}")
                nc.scalar.dma_start(x_tb[:], xb_ext[:, tb])
                x_tbs.append(x_tb)

            # ---- residents ----
            z8 = rpool.tile([128, SCH, H2], fp8)     # fp8 Z, 4KB/part
            sq = rpool.tile([H, T], fp32)            # sigmoid(Q^T)

            # ---- phase A: Z (+colsum) and sigmoid(Q), streaming x ----
            with tc.tile_pool(name="psA", bufs=1, space="PSUM") as psA:
                cs_ps = psA.tile([1, H2], fp32, tag="cs")
                for tb in range(NTB):
                    x_sb = x_tbs[tb]
                    for sl in range(4):
                        s = tb * 4 + sl
                        kv_ps = psA.tile([128, H2], fp32, tag="kv", bufs=2)
                        for d in range(DCH):
                            nc.tensor.matmul(
                                kv_ps[:], x_sb[:, d, sl * 128:(sl + 1) * 128],
                                wkv_sb[:, d, :], start=(d == 0), stop=False)
                        # rank-1 bias fold: += ones^T @ [bv | bk]
                        nc.tensor.matmul(kv_ps[:], ones_sb[:], bkv_sb[:],
                                         start=False, stop=True)
                        ek = wpool.tile([128, H], fp32, tag="ek")
                        nc.scalar.activation(ek[:], kv_ps[:, H:H2], AF.Exp)
                        zb = wpool.tile([128, H2], bf16, tag="zb")
                        nc.vector.tensor_mul(zb[:, 0:H], kv_ps[:, 0:H], ek[:])
                        nc.vector.tensor_copy(zb[:, H:H2], ek[:])
                        # colsum accumulates from high-precision Z
                        nc.tensor.matmul(cs_ps[:], onesc_sb[:], zb[:],
                                         start=(s == 0), stop=(s == SCH - 1))
                        nc.vector.tensor_copy(z8[:, s, :], zb[:])
                    # Q for this t-block; sigmoid via the Exp LUT
                    q_ps = psA.tile([H, 512], fp32, tag="q", bufs=2)
                    for d in range(DCH):
                        nc.tensor.matmul(
                            q_ps[:], wqt_sb[:, d, :],
                            x_sb[:, d, :], start=(d == 0), stop=(d == DCH - 1))
                    eq = wpool.tile([H, 512], fp32, tag="eq")
                    nc.scalar.activation(eq[:], q_ps[:], AF.Exp,
                                         bias=bqn_sb[:], scale=-1.0)
                    nc.vector.tensor_scalar_add(eq[:], eq[:], 1.0)
                    nc.vector.reciprocal_approx_fast(
                        sq[:, tb * 512:(tb + 1) * 512], eq[:])

                # colsum -> hi/lo bf16 pair (keeps full fp32 precision
                # across the two rank-1 matmuls)
                cs_f = wpool.tile([1, H2], fp32, tag="csf", bufs=1)
                nc.scalar.copy(cs_f[:], cs_ps[:])
                cs_hi = wpool.tile([1, H2], bf16, tag="csh", bufs=1)
                nc.vector.tensor_copy(cs_hi[:], cs_f[:])
                cs_lo_f = wpool.tile([1, H2], fp32, tag="cslf", bufs=1)
                nc.vector.tensor_sub(cs_lo_f[:], cs_f[:], cs_hi[:])
                cs_lo = wpool.tile([1, H2], bf16, tag="csl", bufs=1)
                nc.vector.tensor_copy(cs_lo[:], cs_lo_f[:])

            # ---- phase B: nd = SC*(ewm1 @ Z) + SC*colsum ; epilogue ----
            # DoubleRow PSUM outs must sit at partition base 0, so num
            # and den accumulate in separate [64, TQ] tiles.
            with tc.tile_pool(name="psB", bufs=1, space="PSUM") as psB:
                for qt in range(NQT):
                    nd_n = psB.tile([H, TQ], fp32, tag="ndn", bufs=1,
                                    name=f"ndn{qt}")
                    nd_d = psB.tile([H, TQ], fp32, tag="ndd", bufs=1,
                                    name=f"ndd{qt}")
                    for pair in range(NPAIR):
                        ewt = epool.tile([128, 2, TQ], fp8, tag="ew",
                                         bufs=20)
                        nc.sync.dma_start(
                            ewt[:],
                            ewb_ext[qt, pair].rearrange(
                                "p (i t) -> p i t", i=2))
                        for h, nd_ps in ((0, nd_n), (1, nd_d)):
                            lw = z8[:, 2 * pair:2 * pair + 2,
                                    h * 64:(h + 1) * 64]
                            for t2 in range(2):
                                nc.tensor.matmul(
                                    nd_ps[:, t2 * 512:(t2 + 1) * 512],
                                    lw, ewt[:, :, t2 * 512:(t2 + 1) * 512],
                                    start=(pair == 0), stop=False,
                                    perf_mode=DR, skip_group_check=True)
                    # rank-1 colsum correction (hi + lo), closes the groups
                    for h, nd_ps in ((0, nd_n), (1, nd_d)):
                        hs = slice(h * 64, (h + 1) * 64)
                        for t2 in range(2):
                            ts = slice(t2 * 512, (t2 + 1) * 512)
                            nc.tensor.matmul(
                                nd_ps[:, ts], cs_hi[:, hs], scrow_sb[:],
                                start=False, stop=False,
                                skip_group_check=True)
                            nc.tensor.matmul(
                                nd_ps[:, ts], cs_lo[:, hs], scrow_sb[:],
                                start=False, stop=True,
                                skip_group_check=True)
                    # epilogue: yt = sigmoid(Q) * num / den
                    den = wpool.tile([H, TQ], fp32, tag="den", bufs=1,
                                     name=f"den{qt}")
                    nc.scalar.copy(den[:], nd_d[:])
                    rcp = wpool.tile([H, TQ], fp32, tag="rcp", bufs=1,
                                     name=f"rcp{qt}")
                    nc.vector.reciprocal_approx_fast(rcp[:], den[:])
                    r2 = wpool.tile([H, TQ], fp32, tag="r2", bufs=1,
                                    name=f"r2{qt}")
                    nc.vector.tensor_mul(
                        r2[:], rcp[:], sq[:, qt * TQ:(qt + 1) * TQ])
                    yt = wpool.tile([H + 1, TQ], bf16, tag="yt", bufs=2,
                                    name=f"yt{qt}")
                    nc.vector.tensor_mul(yt[0:H, :], nd_n[:], r2[:])
                    nc.vector.memset(yt[H:H + 1, :], 1.0)
                    # output projection + out DMA (scalar ring)
                    for tk in range(TQ // 128):
                        o_ps = psB.tile([128, DIM], fp32, tag="o", bufs=2,
                                        name=f"o_ps{qt}_{tk}")
                        nc.tensor.matmul(
                            o_ps[:], yt[:, tk * 128:(tk + 1) * 128],
                            wpta_sb[:], start=True, stop=True)
                        o_sb = wpool.tile([128, DIM], bf16, tag="o", bufs=4,
                                          name=f"o_sb{qt}_{tk}")
                        if tk % 2 == 0:
                            nc.vector.tensor_copy(o_sb[:], o_ps[:])
                        else:
                            nc.scalar.copy(o_sb[:], o_ps[:])
                        r0 = (qt * (TQ // 128) + tk) * 128
                        nc.scalar.dma_start(out_ext[r0:r0 + 128, :], o_sb[:])

    nc.compile()
    return nc


def _get_nc():
    if "nc" not in _CACHE:
        _CACHE["nc"] = _build()
    return _CACHE["nc"]


def kernel(x, wq, bq, wk, bk, wv, bv, wp, bp, wbias):
    from concourse.bass_utils import run_bass_kernel_spmd

    x = np.asarray(x, dtype=np.float32)
    wbias = np.asarray(wbias, dtype=np.float32)

    # ewm1 pack: mT[s, t] = expm1(wbias[t, s]) * SC, laid out as
    # [qt, pair, p, i, tt] with s = (2*pair + i)*128 + p,
    # t = qt*1024 + tt, so each (qt, pair) DMA is one contiguous
    # [128, 2048B] row block.
    mT = (np.expm1(wbias).T * SC).astype(np.float32)
    ew_pack = np.ascontiguousarray(
        mT.reshape(NPAIR, 2, 128, NQT, TQ).transpose(3, 0, 2, 1, 4)
    ).astype(F8).reshape(NQT, NPAIR, 128, 2 * TQ)

    wkv = np.concatenate([np.asarray(wv).T, np.asarray(wk).T], axis=1)
    wkv_pack = np.ascontiguousarray(
        wkv.reshape(DCH, 128, H2).transpose(1, 0, 2)).astype(BF16)
    wqt_pack = np.ascontiguousarray(
        np.asarray(wq).T.reshape(DCH, 128, H).transpose(1, 0, 2)).astype(BF16)
    wpta = np.concatenate([np.asarray(wp).T, np.asarray(bp)[None, :]],
                          axis=0).astype(BF16)
    bkv = np.concatenate([np.asarray(bv), np.asarray(bk)])[None, :].astype(BF16)
    bqn = (-np.asarray(bq)).reshape(H, 1).astype(np.float32)

    in_maps = []
    for c in range(NCORES):
        # x[c]: [DIM, T] -> [p, tb, d, tt] so each t-block DMA is one
        # contiguous [128, 4KB] row block.
        x_pack = np.ascontiguousarray(
            x[c].reshape(DCH, 128, NTB, 512).transpose(1, 2, 0, 3)
        ).astype(BF16)
        in_maps.append({
            "xb": x_pack, "ewb": ew_pack, "wkv": wkv_pack, "wqt": wqt_pack,
            "wpta": wpta, "bkv": bkv, "bqn": bqn,
        })

    nc = _get_nc()
    res = run_bass_kernel_spmd(nc, in_maps, core_ids=list(range(NCORES)),
                               **RUN_KWARGS)
    LAST_RESULT[0] = res

    out_full = np.empty((B, T, DIM), np.float32)
    for c in range(NCORES):
        out_full[c] = res.results[c]["out"].astype(np.float32)
    return (out_full, out_full)


# revision 13
# speedup vs baseline: 1.4218x; 1.3440x over previous
"""AFT-Full forward on 8 Trainium2 NeuronCores (Bass/Tile, SPMD).

Reference (per batch b):
    Q = x^T wq^T + bq ; K = x^T wk^T + bk ; V = x^T wv^T + bv      # [T, H]
    ew = exp(wbias[:T, :T])                                        # [T, T]
    num = ew @ (exp(K) * V) ; den = ew @ exp(K)                    # [T, H]
    out = (sigmoid(Q) * num / den) @ wp^T + bp                     # [T, DIM]

Sharding: one batch per core (B == NCORES) -- zero collectives.  Each
core loads its full x (bf16, 4MB), streams the full T x T weight matrix
in fp8 (16MB) and writes its out (bf16, 4MB).  The kernel is a pure
DMA-paced stream with no inter-core dependency.

Numerics: ew = exp(wbias) = 1 + expm1(wbias).  The host sends
ewm1 = expm1(wbias)^T * 4096 as float8_e4m3; the rank-1 "ones" part is
applied as colsum = sum_s Z[s,:] computed on-chip from bf16 Z in fp32
and added into the same PSUM accumulation via two bf16 rank-1 matmuls
(hi + lo split of colsum, rhs = a row of 4096.0).  Both operands of the
big matmul are fp8 -> MatmulPerfMode.DoubleRow packs two s-chunks per
instruction (~1.5-2x PE).  Because all the precision-critical mass is
in the colsum term, fp8 quantization of ewm1/Z only perturbs the small
deviation part: CPU-validated end-to-end rel err ~4.0e-3 (the bf16
baseline scheme measures ~4.2e-3).

The num/den ratio cancels the 4096 scale, so no descaling is needed.
Sigmoid is computed as 1/(1+exp(-Q-bq)) on the Exp LUT so the scalar
engine never reloads activation tables.  bkv is folded into the K/V
matmul as a rank-1 accumulation; bp via an appended ones-row in the
output projection.

DMA plan: ew pairs stream on the sync (SP) HWDGE ring in consumption
order; x blocks + out chunks ride the scalar (ACT) HWDGE ring so the
two streams overlap at the HBM controller.
"""

import numpy as np
import ml_dtypes

B, DIM, T, H = 8, 512, 4096, 64
H2 = 2 * H
NCORES = 8
DCH = DIM // 128    # 4 contraction chunks
SCH = T // 128      # 32 s-chunks
NTB = T // 512      # 8 t-blocks for x / Q
NPAIR = SCH // 2    # 16 s-chunk pairs (DoubleRow)
NQT = 4             # t-quarters (DoubleRow PSUM outs must sit at
TQ = T // NQT       # partition base 0 -> separate [64, 1024] num/den)
SC = 4096.0         # fp8 scale for ewm1 (power of 2; cancels in num/den)

BF16 = ml_dtypes.bfloat16
F8 = ml_dtypes.float8_e4m3

_CACHE = {}
RUN_KWARGS = {}        # test harness may set {"trace": True}
LAST_RESULT = [None]   # test harness reads exec_time_ns off this


def _build():
    import concourse.mybir as mybir
    import concourse.tile as tile
    from concourse import bacc

    from concourse.masks import make_identity

    fp32 = mybir.dt.float32
    bf16 = mybir.dt.bfloat16
    fp8 = mybir.dt.float8e4
    AF = mybir.ActivationFunctionType
    DR = mybir.MatmulPerfMode.DoubleRow

    nc = bacc.Bacc("TRN2", target_bir_lowering=False, debug=False,
                   num_devices=NCORES)

    xb_ext = nc.dram_tensor("xb", [128, NTB, DCH, 512], bf16,
                            kind="ExternalInput").ap()
    ewb_ext = nc.dram_tensor("ewb", [NQT, NPAIR, 128, 2 * TQ], fp8,
                             kind="ExternalInput").ap()
    wkv_ext = nc.dram_tensor("wkv", [128, DCH, H2], bf16,
                             kind="ExternalInput").ap()
    wqt_ext = nc.dram_tensor("wqt", [128, DCH, H], bf16,
                             kind="ExternalInput").ap()
    wpta_ext = nc.dram_tensor("wpta", [H + 1, DIM], bf16,
                              kind="ExternalInput").ap()
    bkv_ext = nc.dram_tensor("bkv", [1, H2], bf16, kind="ExternalInput").ap()
    bqn_ext = nc.dram_tensor("bqn", [H, 1], fp32, kind="ExternalInput").ap()
    out_ext = nc.dram_tensor("out", [T, DIM], bf16, kind="ExternalOutput").ap()

    with tile.TileContext(nc) as tc:
        with (
            tc.tile_pool(name="const", bufs=1) as cpool,
            tc.tile_pool(name="res", bufs=1) as rpool,
            tc.tile_pool(name="work", bufs=2) as wpool,
            tc.tile_pool(name="ew", bufs=1) as epool,
        ):
            # ---- constants (sync ring; tiny) ----
            wkv_sb = cpool.tile([128, DCH, H2], bf16)
            nc.sync.dma_start(wkv_sb[:], wkv_ext[:])
            wqt_sb = cpool.tile([128, DCH, H], bf16)
            nc.sync.dma_start(wqt_sb[:], wqt_ext[:])
            wpta_sb = cpool.tile([H + 1, DIM], bf16)
            nc.sync.dma_start(wpta_sb[:], wpta_ext[:])
            bkv_sb = cpool.tile([1, H2], bf16)
            nc.sync.dma_start(bkv_sb[:], bkv_ext[:])
            bqn_sb = cpool.tile([H, 1], fp32)
            nc.sync.dma_start(bqn_sb[:], bqn_ext[:])
            ones512 = cpool.tile([1, 512], bf16)     # bkv rank-1 rhs
            nc.vector.memset(ones512[:], 1.0)
            id_sb = cpool.tile([128, 128], bf16)     # PE-transpose identity
            make_identity(nc, id_sb[:])

            # ---- x blocks on the scalar ring (overlap the ew stream) ----
            x_tbs = []
            for tb in range(NTB):
                x_tb = rpool.tile([128, DCH, 512], bf16, name=f"x{tb}")
                nc.scalar.dma_start(x_tb[:], xb_ext[:, tb])
                x_tbs.append(x_tb)

            # ---- residents ----
            z8 = rpool.tile([128, SCH, H2], fp8)     # fp8 Z, 4KB/part
            sq = rpool.tile([H, T], fp32)            # sigmoid(Q^T)

            # ---- phase A: Z (+colsum) and sigmoid(Q), streaming x ----
            # kv is computed in [H2, t] orientation (moving = x, 512-wide
            # fills) and transposed back to [s, H2] on the PE; colsum is a
            # free-dim DVE reduction in this orientation.  The transposes
            # for block tb run one iteration later so the ACT/DVE chain
            # producing zbt never stalls the PE.
            with tc.tile_pool(name="psA", bufs=1, space="PSUM") as psA:
                cs_parts = rpool.tile([H2, NTB], fp32)
                zbts = [None] * NTB

                def transposes(tb):
                    zbt = zbts[tb]
                    for sl in range(4):
                        tr_ps = psA.tile([128, 128], bf16, tag="tr", bufs=3,
                                         name=f"tr{tb}_{sl}")
                        nc.tensor.transpose(
                            tr_ps[:], zbt[:, sl * 128:(sl + 1) * 128],
                            id_sb[:])
                        nc.vector.tensor_copy(z8[:, tb * 4 + sl, :], tr_ps[:])

                for tb in range(NTB):
                    x_sb = x_tbs[tb]
                    kv_ps = psA.tile([H2, 512], fp32, tag="kv", bufs=2)
                    for d in range(DCH):
                        nc.tensor.matmul(
                            kv_ps[:], wkv_sb[:, d, :], x_sb[:, d, :],
                            start=(d == 0), stop=False)
                    # rank-1 bias fold: += [bv | bk]^T @ ones
                    nc.tensor.matmul(kv_ps[:], bkv_sb[:], ones512[:],
                                     start=False, stop=True)
                    ek = wpool.tile([H, 512], fp32, tag="ek")
                    nc.scalar.activation(ek[:], kv_ps[H:H2, :], AF.Exp)
                    zbt = wpool.tile([H2, 512], bf16, tag="zbt", bufs=2,
                                     name=f"zbt{tb}")
                    nc.vector.tensor_mul(zbt[0:H, :], kv_ps[0:H, :], ek[:])
                    nc.vector.tensor_copy(zbt[H:H2, :], ek[:])
                    nc.vector.reduce_sum(cs_parts[:, tb:tb + 1], zbt[:],
                                         axis=mybir.AxisListType.X)
                    zbts[tb] = zbt
                    # Q for this t-block; sigmoid via the Exp LUT
                    q_ps = psA.tile([H, 512], fp32, tag="q", bufs=2)
                    for d in range(DCH):
                        nc.tensor.matmul(
                            q_ps[:], wqt_sb[:, d, :],
                            x_sb[:, d, :], start=(d == 0), stop=(d == DCH - 1))
                    eq = wpool.tile([H, 512], fp32, tag="eq")
                    nc.scalar.activation(eq[:], q_ps[:], AF.Exp,
                                         bias=bqn_sb[:], scale=-1.0)
                    nc.vector.tensor_scalar_add(eq[:], eq[:], 1.0)
                    nc.vector.reciprocal_approx_fast(
                        sq[:, tb * 512:(tb + 1) * 512], eq[:])
                    if tb >= 1:
                        transposes(tb - 1)
                transposes(NTB - 1)

                # colsum -> per-partition fp32 bias vectors (x SC), both
                # halves moved to partition base 0 for the ACT bias adds
                cs_raw = wpool.tile([H2, 1], fp32, tag="csr", bufs=1)
                nc.vector.reduce_sum(cs_raw[:], cs_parts[:],
                                     axis=mybir.AxisListType.X)
                cs_num = wpool.tile([H, 1], fp32, tag="csn", bufs=1)
                nc.vector.tensor_scalar_mul(cs_num[:], cs_raw[0:H, :], SC)
                cs_den_r = wpool.tile([H, 1], fp32, tag="csdr", bufs=1)
                nc.sync.dma_start(cs_den_r[:], cs_raw[H:H2, :])
                cs_den = wpool.tile([H, 1], fp32, tag="csd", bufs=1)
                nc.vector.tensor_scalar_mul(cs_den[:], cs_den_r[:], SC)

            # ---- phase B: nd = SC*(ewm1 @ Z) + SC*colsum ; epilogue ----
            # Plain fp8 matmuls ([128, 512] outs, FWL active) run at the
            # same MAC rate as DoubleRow without its LDWEIGHTS penalty.
            with tc.tile_pool(name="psB", bufs=1, space="PSUM") as psB:
                for qt in range(NQT):
                    nd_ps = psB.tile([H2, TQ], fp32, tag="nd", bufs=2,
                                     name=f"nd{qt}")
                    for pair in range(NPAIR):
                        ewt = epool.tile([128, 2, TQ], fp8, tag="ew",
                                         bufs=20)
                        nc.sync.dma_start(
                            ewt[:],
                            ewb_ext[qt, pair].rearrange(
                                "p (i t) -> p i t", i=2))
                        for i in range(2):
                            s = 2 * pair + i
                            for t2 in range(2):
                                nc.tensor.matmul(
                                    nd_ps[:, t2 * 512:(t2 + 1) * 512],
                                    z8[:, s, :],
                                    ewt[:, i, t2 * 512:(t2 + 1) * 512],
                                    start=(s == 0), stop=(s == SCH - 1))
                    # epilogue: yt = sigmoid(Q) * num / den, with the
                    # SC*colsum correction folded in as per-partition
                    # scalar adds (den half is moved to partition base 0
                    # by the ACT copy first)
                    den = wpool.tile([H, TQ], fp32, tag="den", bufs=1,
                                     name=f"den{qt}")
                    nc.scalar.copy(den[:], nd_ps[H:H2, :])
                    nc.vector.tensor_scalar_add(den[:], den[:], cs_den[:])
                    rcp = wpool.tile([H, TQ], fp32, tag="rcp", bufs=1,
                                     name=f"rcp{qt}")
                    nc.vector.reciprocal_approx_fast(rcp[:], den[:])
                    r2 = wpool.tile([H, TQ], fp32, tag="r2", bufs=1,
                                    name=f"r2{qt}")
                    nc.vector.tensor_mul(
                        r2[:], rcp[:], sq[:, qt * TQ:(qt + 1) * TQ])
                    yt = wpool.tile([H + 1, TQ], bf16, tag="yt", bufs=2,
                                    name=f"yt{qt}")
                    nc.vector.scalar_tensor_tensor(
                        yt[0:H, :], nd_ps[0:H, :], cs_num[:], r2[:],
                        mybir.AluOpType.add, mybir.AluOpType.mult)
                    nc.vector.memset(yt[H:H + 1, :], 1.0)
                    # output projection + out DMA (scalar ring)
                    for tk in range(TQ // 128):
                        o_ps = psB.tile([128, DIM], fp32, tag="o", bufs=2,
                                        name=f"o_ps{qt}_{tk}")
                        nc.tensor.matmul(
                            o_ps[:], yt[:, tk * 128:(tk + 1) * 128],
                            wpta_sb[:], start=True, stop=True)
                        o_sb = wpool.tile([128, DIM], bf16, tag="o", bufs=4,
                                          name=f"o_sb{qt}_{tk}")
                        if tk % 2 == 0:
                            nc.vector.tensor_copy(o_sb[:], o_ps[:])
                        else:
                            nc.scalar.copy(o_sb[:], o_ps[:])
                        r0 = (qt * (TQ // 128) + tk) * 128
                        nc.scalar.dma_start(out_ext[r0:r0 + 128, :], o_sb[:])

    nc.compile()
    return nc


def _get_nc():
    if "nc" not in _CACHE:
        _CACHE["nc"] = _build()
    return _CACHE["nc"]


def kernel(x, wq, bq, wk, bk, wv, bv, wp, bp, wbias):
    from concourse.bass_utils import run_bass_kernel_spmd

    x = np.asarray(x, dtype=np.float32)
    wbias = np.asarray(wbias, dtype=np.float32)

    # ewm1 pack: mT[s, t] = expm1(wbias[t, s]) * SC, laid out as
    # [qt, pair, p, i, tt] with s = (2*pair + i)*128 + p,
    # t = qt*1024 + tt, so each (qt, pair) DMA is one contiguous
    # [128, 2048B] row block.
    mT = (np.expm1(wbias).T * SC).astype(np.float32)
    ew_pack = np.ascontiguousarray(
        mT.reshape(NPAIR, 2, 128, NQT, TQ).transpose(3, 0, 2, 1, 4)
    ).astype(F8).reshape(NQT, NPAIR, 128, 2 * TQ)

    wkv = np.concatenate([np.asarray(wv).T, np.asarray(wk).T], axis=1)
    wkv_pack = np.ascontiguousarray(
        wkv.reshape(DCH, 128, H2).transpose(1, 0, 2)).astype(BF16)
    wqt_pack = np.ascontiguousarray(
        np.asarray(wq).T.reshape(DCH, 128, H).transpose(1, 0, 2)).astype(BF16)
    wpta = np.concatenate([np.asarray(wp).T, np.asarray(bp)[None, :]],
                          axis=0).astype(BF16)
    bkv = np.concatenate([np.asarray(bv), np.asarray(bk)])[None, :].astype(BF16)
    bqn = (-np.asarray(bq)).reshape(H, 1).astype(np.float32)

    in_maps = []
    for c in range(NCORES):
        # x[c]: [DIM, T] -> [p, tb, d, tt] so each t-block DMA is one
        # contiguous [128, 4KB] row block.
        x_pack = np.ascontiguousarray(
            x[c].reshape(DCH, 128, NTB, 512).transpose(1, 2, 0, 3)
        ).astype(BF16)
        in_maps.append({
            "xb": x_pack, "ewb": ew_pack, "wkv": wkv_pack, "wqt": wqt_pack,
            "wpta": wpta, "bkv": bkv, "bqn": bqn,
        })

    nc = _get_nc()
    res = run_bass_kernel_spmd(nc, in_maps, core_ids=list(range(NCORES)),
                               **RUN_KWARGS)
    LAST_RESULT[0] = res

    out_full = np.empty((B, T, DIM), np.float32)
    for c in range(NCORES):
        out_full[c] = res.results[c]["out"].astype(np.float32)
    return (out_full, out_full)


# revision 18
# speedup vs baseline: 1.6029x; 1.1274x over previous
"""AFT-Full forward on 8 Trainium2 NeuronCores (Bass/Tile, SPMD).

Reference (per batch b):
    Q = x^T wq^T + bq ; K = x^T wk^T + bk ; V = x^T wv^T + bv      # [T, H]
    ew = exp(wbias[:T, :T])                                        # [T, T]
    num = ew @ (exp(K) * V) ; den = ew @ exp(K)                    # [T, H]
    out = (sigmoid(Q) * num / den) @ wp^T + bp                     # [T, DIM]

Sharding: one batch per core (B == NCORES) -- zero collectives.  Each
core loads its full x (bf16, 4MB), streams the full T x T weight matrix
in fp8 (16MB) and writes its out (bf16, 4MB).  The kernel is a pure
DMA-paced stream with no inter-core dependency.

Numerics: ew = exp(wbias) = 1 + expm1(wbias).  The host sends
ewm1 = expm1(wbias)^T * 4096 as float8_e4m3; the rank-1 "ones" part is
applied as colsum = sum_s Z[s,:] computed on-chip from bf16 Z in fp32
and added into the same PSUM accumulation via two bf16 rank-1 matmuls
(hi + lo split of colsum, rhs = a row of 4096.0).  Both operands of the
big matmul are fp8 -> MatmulPerfMode.DoubleRow packs two s-chunks per
instruction (~1.5-2x PE).  Because all the precision-critical mass is
in the colsum term, fp8 quantization of ewm1/Z only perturbs the small
deviation part: CPU-validated end-to-end rel err ~4.0e-3 (the bf16
baseline scheme measures ~4.2e-3).

The num/den ratio cancels the 4096 scale, so no descaling is needed.
Sigmoid is computed as 1/(1+exp(-Q-bq)) on the Exp LUT so the scalar
engine never reloads activation tables.  bkv is folded into the K/V
matmul as a rank-1 accumulation; bp via an appended ones-row in the
output projection.

DMA plan: ew pairs stream on the sync (SP) HWDGE ring in consumption
order; x blocks + out chunks ride the scalar (ACT) HWDGE ring so the
two streams overlap at the HBM controller.
"""

import numpy as np
import ml_dtypes

B, DIM, T, H = 8, 512, 4096, 64
H2 = 2 * H
NCORES = 8
DCH = DIM // 128    # 4 contraction chunks
SCH = T // 128      # 32 s-chunks
NTB = T // 512      # 8 t-blocks for x / Q
NPAIR = SCH // 2    # 16 s-chunk pairs (DoubleRow)
NQT = 4             # t-quarters (DoubleRow PSUM outs must sit at
TQ = T // NQT       # partition base 0 -> separate [64, 1024] num/den)
SC = 4096.0         # fp8 scale for ewm1 (power of 2; cancels in num/den)

BF16 = ml_dtypes.bfloat16
F8 = ml_dtypes.float8_e4m3

_CACHE = {}
RUN_KWARGS = {}        # test harness may set {"trace": True}
LAST_RESULT = [None]   # test harness reads exec_time_ns off this


def _build():
    import concourse.mybir as mybir
    import concourse.tile as tile
    from concourse import bacc

    from concourse.masks import make_identity

    fp32 = mybir.dt.float32
    bf16 = mybir.dt.bfloat16
    fp8 = mybir.dt.float8e4
    AF = mybir.ActivationFunctionType
    DR = mybir.MatmulPerfMode.DoubleRow

    nc = bacc.Bacc("TRN2", target_bir_lowering=False, debug=False,
                   num_devices=NCORES)

    xb_ext = nc.dram_tensor("xb", [128, NTB, DCH, 512], bf16,
                            kind="ExternalInput").ap()
    ewb_ext = nc.dram_tensor("ewb", [NQT, NPAIR, 128, 2 * TQ], fp8,
                             kind="ExternalInput").ap()
    wkv_ext = nc.dram_tensor("wkv", [128, DCH, H2], bf16,
                             kind="ExternalInput").ap()
    wqt_ext = nc.dram_tensor("wqt", [128, DCH, H], bf16,
                             kind="ExternalInput").ap()
    wpta_ext = nc.dram_tensor("wpta", [H + 1, DIM], bf16,
                              kind="ExternalInput").ap()
    bkv_ext = nc.dram_tensor("bkv", [1, H2], bf16, kind="ExternalInput").ap()
    bqn_ext = nc.dram_tensor("bqn", [H, 1], fp32, kind="ExternalInput").ap()
    out_ext = nc.dram_tensor("out", [T, DIM], bf16, kind="ExternalOutput").ap()

    with tile.TileContext(nc) as tc:
        with (
            tc.tile_pool(name="const", bufs=1) as cpool,
            tc.tile_pool(name="res", bufs=1) as rpool,
            tc.tile_pool(name="work", bufs=2) as wpool,
            tc.tile_pool(name="ew", bufs=1) as epool,
        ):
            # ---- constants (sync ring; tiny) ----
            wkv_sb = cpool.tile([128, DCH, H2], bf16)
            nc.sync.dma_start(wkv_sb[:], wkv_ext[:])
            wqt_sb = cpool.tile([128, DCH, H], bf16)
            nc.sync.dma_start(wqt_sb[:], wqt_ext[:])
            wpta_sb = cpool.tile([H + 1, DIM], bf16)
            nc.sync.dma_start(wpta_sb[:], wpta_ext[:])
            bkv_sb = cpool.tile([1, H2], bf16)
            nc.sync.dma_start(bkv_sb[:], bkv_ext[:])
            bqn_sb = cpool.tile([H, 1], fp32)
            nc.sync.dma_start(bqn_sb[:], bqn_ext[:])
            ones512 = cpool.tile([1, 512], bf16)     # bkv rank-1 rhs
            nc.vector.memset(ones512[:], 1.0)
            id_sb = cpool.tile([128, 128], bf16)     # PE-transpose identity
            make_identity(nc, id_sb[:])

            # ---- x blocks head the sync ring (ew queues behind them);
            # the scalar ring carries only the out writes ----
            x_tbs = []
            for tb in range(NTB):
                x_tb = rpool.tile([128, DCH, 512], bf16, name=f"x{tb}")
                nc.sync.dma_start(x_tb[:], xb_ext[:, tb])
                x_tbs.append(x_tb)

            # ---- residents ----
            z8 = rpool.tile([128, SCH, H2], fp8)     # fp8 Z, 4KB/part
            sq = rpool.tile([H, T], fp32)            # sigmoid(Q^T)

            # ---- phase A: Z (+colsum) and sigmoid(Q), streaming x ----
            # kv is computed in [H2, t] orientation (moving = x, 512-wide
            # fills) and transposed back to [s, H2] on the PE; colsum is a
            # free-dim DVE reduction in this orientation.  The transposes
            # for block tb run one iteration later so the ACT/DVE chain
            # producing zbt never stalls the PE.
            with tc.tile_pool(name="psA", bufs=1, space="PSUM") as psA:
                cs_parts = rpool.tile([H2, NTB], fp32)
                zbts = [None] * NTB

                def transposes(tb):
                    zbt = zbts[tb]
                    for sl in range(4):
                        tr_ps = psA.tile([128, 128], bf16, tag="tr", bufs=3,
                                         name=f"tr{tb}_{sl}")
                        nc.tensor.transpose(
                            tr_ps[:], zbt[:, sl * 128:(sl + 1) * 128],
                            id_sb[:])
                        nc.scalar.copy(z8[:, tb * 4 + sl, :], tr_ps[:])

                for tb in range(NTB):
                    x_sb = x_tbs[tb]
                    kv_ps = psA.tile([H2, 512], fp32, tag="kv", bufs=2)
                    for d in range(DCH):
                        nc.tensor.matmul(
                            kv_ps[:], wkv_sb[:, d, :], x_sb[:, d, :],
                            start=(d == 0), stop=False)
                    # rank-1 bias fold: += [bv | bk]^T @ ones
                    nc.tensor.matmul(kv_ps[:], bkv_sb[:], ones512[:],
                                     start=False, stop=True)
                    zbt = wpool.tile([H2, 512], bf16, tag="zbt", bufs=2,
                                     name=f"zbt{tb}")
                    nc.scalar.activation(zbt[H:H2, :], kv_ps[H:H2, :], AF.Exp)
                    nc.vector.tensor_mul(zbt[0:H, :], kv_ps[0:H, :],
                                         zbt[H:H2, :])
                    nc.vector.reduce_sum(cs_parts[:, tb:tb + 1], zbt[:],
                                         axis=mybir.AxisListType.X)
                    zbts[tb] = zbt
                    # Q for this t-block; sigmoid via the Exp LUT
                    q_ps = psA.tile([H, 512], fp32, tag="q", bufs=2)
                    for d in range(DCH):
                        nc.tensor.matmul(
                            q_ps[:], wqt_sb[:, d, :],
                            x_sb[:, d, :], start=(d == 0), stop=(d == DCH - 1))
                    eq = wpool.tile([H, 512], fp32, tag="eq")
                    nc.scalar.activation(eq[:], q_ps[:], AF.Exp,
                                         bias=bqn_sb[:], scale=-1.0)
                    nc.vector.tensor_scalar_add(eq[:], eq[:], 1.0)
                    nc.vector.reciprocal_approx_fast(
                        sq[:, tb * 512:(tb + 1) * 512], eq[:])
                    if tb >= 1:
                        transposes(tb - 1)
                transposes(NTB - 1)

                # colsum -> per-partition fp32 bias vectors (x SC), both
                # halves moved to partition base 0 for the ACT bias adds
                cs_raw = wpool.tile([H2, 1], fp32, tag="csr", bufs=1)
                nc.vector.reduce_sum(cs_raw[:], cs_parts[:],
                                     axis=mybir.AxisListType.X)
                cs_num = wpool.tile([H, 1], fp32, tag="csn", bufs=1)
                nc.vector.tensor_scalar_mul(cs_num[:], cs_raw[0:H, :], SC)
                # partition shift via SWDGE so it doesn't queue behind the
                # ew stream on the sync ring
                cs_den_r = wpool.tile([H, 1], fp32, tag="csdr", bufs=1)
                nc.gpsimd.dma_start(cs_den_r[:], cs_raw[H:H2, :])
                cs_den = wpool.tile([H, 1], fp32, tag="csd", bufs=1)
                nc.vector.tensor_scalar_mul(cs_den[:], cs_den_r[:], SC)

            # ---- phase B: nd = SC*(ewm1 @ Z) + SC*colsum ; epilogue ----
            # Plain fp8 matmuls ([128, 512] outs, FWL active) run at the
            # same MAC rate as DoubleRow without its LDWEIGHTS penalty.
            with tc.tile_pool(name="psB", bufs=1, space="PSUM") as psB:
                for qt in range(NQT):
                    nd_ps = psB.tile([H2, TQ], fp32, tag="nd", bufs=2,
                                     name=f"nd{qt}")
                    for pair in range(NPAIR):
                        ewt = epool.tile([128, 2, TQ], fp8, tag="ew",
                                         bufs=20)
                        nc.sync.dma_start(
                            ewt[:],
                            ewb_ext[qt, pair].rearrange(
                                "p (i t) -> p i t", i=2))
                        for i in range(2):
                            s = 2 * pair + i
                            for t2 in range(2):
                                nc.tensor.matmul(
                                    nd_ps[:, t2 * 512:(t2 + 1) * 512],
                                    z8[:, s, :],
                                    ewt[:, i, t2 * 512:(t2 + 1) * 512],
                                    start=(s == 0), stop=(s == SCH - 1))
                    # epilogue: yt = sigmoid(Q) * num / den, with the
                    # SC*colsum correction folded in as per-partition
                    # scalar adds (den half is moved to partition base 0
                    # by the ACT copy first).  Split into 512-wide
                    # sub-blocks so the chain latency is short and the
                    # oproj matmuls start early.
                    yt = wpool.tile([H + 1, TQ], bf16, tag="yt", bufs=2,
                                    name=f"yt{qt}")
                    for eb in range(2):
                        es = slice(eb * 512, (eb + 1) * 512)
                        den = wpool.tile([H, 512], fp32, tag="den", bufs=2,
                                         name=f"den{qt}_{eb}")
                        nc.scalar.copy(den[:], nd_ps[H:H2, es])
                        nc.vector.tensor_scalar_add(den[:], den[:],
                                                    cs_den[:])
                        rcp = wpool.tile([H, 512], fp32, tag="rcp", bufs=2,
                                         name=f"rcp{qt}_{eb}")
                        nc.vector.reciprocal_approx_fast(rcp[:], den[:])
                        r2 = wpool.tile([H, 512], fp32, tag="r2", bufs=2,
                                        name=f"r2{qt}_{eb}")
                        q0 = qt * TQ + eb * 512
                        nc.vector.tensor_mul(r2[:], rcp[:],
                                             sq[:, q0:q0 + 512])
                        nc.vector.scalar_tensor_tensor(
                            yt[0:H, es], nd_ps[0:H, es], cs_num[:], r2[:],
                            mybir.AluOpType.add, mybir.AluOpType.mult)
                        nc.vector.memset(yt[H:H + 1, es], 1.0)
                        # output projection + out DMA (scalar ring)
                        for tk in range(4):
                            tkk = eb * 4 + tk
                            o_ps = psB.tile([128, DIM], fp32, tag="o",
                                            bufs=2, name=f"o_ps{qt}_{tkk}")
                            nc.tensor.matmul(
                                o_ps[:], yt[:, tkk * 128:(tkk + 1) * 128],
                                wpta_sb[:], start=True, stop=True)
                            o_sb = wpool.tile([128, DIM], bf16, tag="o",
                                              bufs=4, name=f"o_sb{qt}_{tkk}")
                            if tkk % 2 == 0:
                                nc.vector.tensor_copy(o_sb[:], o_ps[:])
                            else:
                                nc.scalar.copy(o_sb[:], o_ps[:])
                            r0 = (qt * (TQ // 128) + tkk) * 128
                            nc.scalar.dma_start(out_ext[r0:r0 + 128, :],
                                                o_sb[:])

    nc.compile()
    return nc


def _get_nc():
    if "nc" not in _CACHE:
        _CACHE["nc"] = _build()
    return _CACHE["nc"]


def kernel(x, wq, bq, wk, bk, wv, bv, wp, bp, wbias):
    from concourse.bass_utils import run_bass_kernel_spmd

    x = np.asarray(x, dtype=np.float32)
    wbias = np.asarray(wbias, dtype=np.float32)

    # ewm1 pack: mT[s, t] = expm1(wbias[t, s]) * SC, laid out as
    # [qt, pair, p, i, tt] with s = (2*pair + i)*128 + p,
    # t = qt*1024 + tt, so each (qt, pair) DMA is one contiguous
    # [128, 2048B] row block.
    mT = (np.expm1(wbias).T * SC).astype(np.float32)
    ew_pack = np.ascontiguousarray(
        mT.reshape(NPAIR, 2, 128, NQT, TQ).transpose(3, 0, 2, 1, 4)
    ).astype(F8).reshape(NQT, NPAIR, 128, 2 * TQ)

    wkv = np.concatenate([np.asarray(wv).T, np.asarray(wk).T], axis=1)
    wkv_pack = np.ascontiguousarray(
        wkv.reshape(DCH, 128, H2).transpose(1, 0, 2)).astype(BF16)
    wqt_pack = np.ascontiguousarray(
        np.asarray(wq).T.reshape(DCH, 128, H).transpose(1, 0, 2)).astype(BF16)
    wpta = np.concatenate([np.asarray(wp).T, np.asarray(bp)[None, :]],
                          axis=0).astype(BF16)
    bkv = np.concatenate([np.asarray(bv), np.asarray(bk)])[None, :].astype(BF16)
    bqn = (-np.asarray(bq)).reshape(H, 1).astype(np.float32)

    in_maps = []
    for c in range(NCORES):
        # x[c]: [DIM, T] -> [p, tb, d, tt] so each t-block DMA is one
        # contiguous [128, 4KB] row block.
        x_pack = np.ascontiguousarray(
            x[c].reshape(DCH, 128, NTB, 512).transpose(1, 2, 0, 3)
        ).astype(BF16)
        in_maps.append({
            "xb": x_pack, "ewb": ew_pack, "wkv": wkv_pack, "wqt": wqt_pack,
            "wpta": wpta, "bkv": bkv, "bqn": bqn,
        })

    nc = _get_nc()
    res = run_bass_kernel_spmd(nc, in_maps, core_ids=list(range(NCORES)),
                               **RUN_KWARGS)
    LAST_RESULT[0] = res

    out_full = np.empty((B, T, DIM), np.float32)
    for c in range(NCORES):
        out_full[c] = res.results[c]["out"].astype(np.float32)
    return (out_full, out_full)
